# revision 30
# baseline (speedup 1.0000x reference)
"""Trainium2 Bass kernel for nn_CategoricalEncoder (vq_codebook).

Computes: logits = x.reshape(B,T,N,S); idx = argmax(logits + gumbel(key42));
out[b,t,n,:] = codebook[n, idx[b,t,n], :]  (the straight-through softmax terms
cancel numerically to ~1e-7, so the exact one-hot @ codebook matmul suffices).

The Gumbel noise is a fixed constant (key 42, fixed shape/dtype); it is
precomputed on the host with the same jax backend the reference uses and
streamed to the device, where argmax/one-hot/codebook-matmul run.

Sharding: data-parallel over batch B across the 8 NeuronCores; codebook
replicated.
"""

import numpy as np

B, T, N, S, E = 32, 256, 32, 32, 128
NCORES = 8
TOK = (B // NCORES) * T  # tokens per core (1024)
PTILE = 128
NTILES = TOK // PTILE  # 8
NCHUNK = 8  # (n,s) chunks of 128 per token-tile; each covers 4 n's

_cache: dict = {}


def _gumbel() -> np.ndarray:
    """Gumbel(0,1) noise bits exactly as jax.random.categorical(key(42), ...)
    draws them on this process's default jax backend."""
    if "g" not in _cache:
        import jax
        import jax.numpy as jnp

        g = jax.random.gumbel(jax.random.key(42), (B, T, N, S), jnp.float32)
        _cache["g"] = np.asarray(g).reshape(B, T, N * S)
    return _cache["g"]


def _build_bass():
    if "nc" in _cache:
        return _cache["nc"]
    from contextlib import ExitStack

    import concourse.bacc as bacc
    import concourse.bass as bass
    import concourse.tile as tile
    from concourse import mybir
    from concourse.masks import make_identity

    fp32 = mybir.dt.float32
    bf16 = mybir.dt.bfloat16
    nc = bacc.Bacc("TRN2", target_bir_lowering=False)
    x = nc.declare_dram_parameter("x", [TOK, N * S], fp32, isOutput=False)
    g = nc.declare_dram_parameter("g", [TOK, N * S], fp32, isOutput=False)
    cbd = nc.declare_dram_parameter("cbd", [2, 128, NCHUNK, 512], bf16, isOutput=False)
    out = nc.declare_dram_parameter("out", [TOK, N * E], fp32, isOutput=True)

    with ExitStack() as ctx:
        tc = ctx.enter_context(tile.TileContext(nc))
        singles = ctx.enter_context(tc.tile_pool(name="singles", bufs=1))
        ins = ctx.enter_context(tc.tile_pool(name="ins", bufs=4))
        work = ctx.enter_context(tc.tile_pool(name="work", bufs=3))
        oht_pool = ctx.enter_context(tc.tile_pool(name="oht", bufs=4))
        stage_pool = ctx.enter_context(tc.tile_pool(name="stage", bufs=2))
        pst = ctx.enter_context(tc.tile_pool(name="pst", bufs=2, space="PSUM"))
        pso = ctx.enter_context(tc.tile_pool(name="pso", bufs=3, space="PSUM"))

        identity = singles.tile([128, 128], fp32, tag="identity")
        make_identity(nc, identity)

        # Block-diagonal codebook: chunk c is [(4n x 32s)=128, (4n x 128e)=512]
        # so a single K=128 matmul contracts 4 consecutive n's at once.
        # bf16 hi+lo split: out = onehot @ hi + onehot @ lo (error ~2^-18).
        # Loaded via gpsimd (SWDGE) so the SP HWDGE ring starts with x/g.
        bdcb = []
        cbd_insts = []
        for h in range(2):
            t_ = singles.tile([128, NCHUNK, 512], bf16, tag=f"bdcb{h}")
            cbd_insts.append(nc.gpsimd.dma_start(out=t_, in_=cbd[h]).ins)
            bdcb.append(t_)

        # Chain input DMAs in consumption order so tile0's data lands first
        # at full HBM bandwidth instead of all transfers time-sharing it.
        in_chain = []

        for it in range(NTILES):
            rows = slice(it * PTILE, (it + 1) * PTILE)
            x_t = ins.tile([PTILE, N * S], fp32, tag="x")
            g_t = ins.tile([PTILE, N * S], fp32, tag="g")
            in_chain.append(nc.sync.dma_start(out=x_t, in_=x[rows]).ins)
            in_chain.append(nc.sync.dma_start(out=g_t, in_=g[rows]).ins)
            if it == 0:
                # codebook transfers slot in right after tile0's inputs
                in_chain.extend(cbd_insts)

            v = work.tile([PTILE, N * S], fp32, tag="v")
            nc.vector.tensor_tensor(out=v, in0=x_t, in1=g_t, op=mybir.AluOpType.add)
            v3 = v.rearrange("p (n s) -> p n s", s=S)

            m = work.tile([PTILE, N], fp32, tag="m")
            nc.vector.tensor_reduce(
                out=m, in_=v3, axis=mybir.AxisListType.X, op=mybir.AluOpType.max
            )

            onehot = work.tile([PTILE, N * S], fp32, tag="onehot")
            m_b = m.unsqueeze(2).broadcast_to([PTILE, N, S])
            nc.vector.tensor_tensor(
                out=onehot.rearrange("p (n s) -> p n s", s=S),
                in0=v3,
                in1=m_b,
                op=mybir.AluOpType.is_ge,
            )

            stage = stage_pool.tile([PTILE, N * E], fp32, tag="stage")
            # 4 transposes share one PSUM bank -> single ACT copy (fp32->bf16)
            ohTs = []
            for q in range(2):
                pst_t = pst.tile([128, 512], fp32, tag="pst")
                for c4 in range(4):
                    c = 4 * q + c4
                    nc.tensor.transpose(
                        pst_t[:, c4 * 128 : (c4 + 1) * 128],
                        onehot[:, c * 128 : (c + 1) * 128],
                        identity,
                    )
                ohT = oht_pool.tile([128, 512], bf16, tag="ohT")
                nc.scalar.copy(out=ohT, in_=pst_t)
                ohTs.append(ohT)
            for p2 in range(4):  # pairs of chunks -> one 2-bank psum tile
                pso_t = pso.tile([128, 1024], fp32, tag="pso")
                for ci in range(2):
                    c = 2 * p2 + ci
                    ohT_c = ohTs[c // 4][:, (c % 4) * 128 : (c % 4 + 1) * 128]
                    ps = pso_t[:, ci * 512 : (ci + 1) * 512]
                    nc.tensor.matmul(ps, ohT_c, bdcb[0][:, c, :], start=True, stop=False)
                    nc.tensor.matmul(ps, ohT_c, bdcb[1][:, c, :], start=False, stop=True)
                # Split PSUM->SBUF evacuation between DVE and ACT
                eng_dve = ((it * 4 + p2) * 5) % 16 < 5  # ~5/16 of copies on DVE
                dst = stage[:, p2 * 1024 : (p2 + 1) * 1024]
                if eng_dve:
                    nc.vector.tensor_copy(out=dst, in_=pso_t)
                else:
                    nc.scalar.copy(out=dst, in_=pso_t)
                if p2 == 1:
                    nc.scalar.dma_start(
                        out=out[rows, :2048], in_=stage[:, :2048]
                    )
            # Output store on the ACT HWDGE ring so it doesn't FIFO-serialize
            # with the input loads on the SP ring.
            nc.scalar.dma_start(out=out[rows, 2048:], in_=stage[:, 2048:])

        # Stride-3 chains: ~3 input transfers in flight — early tiles get
        # most of the HBM bandwidth, completion latency stays hidden.
        for j, inst_ in enumerate(in_chain):
            tc.chain_iter_dep(f"in_dma_chain{j % 3}", inst_)

    nc.finalize()
    _cache["nc"] = nc
    return nc


def _blockdiag_cb(cb: np.ndarray) -> np.ndarray:
    """[N,S,E] -> [2, 128, NCHUNK, 512] bf16 block-diagonal hi/lo tiles."""
    import ml_dtypes

    bf16 = ml_dtypes.bfloat16
    full = np.zeros((NCHUNK, 128, 512), dtype=np.float32)
    for c in range(NCHUNK):
        for nl in range(4):
            full[c, nl * 32 : (nl + 1) * 32, nl * 128 : (nl + 1) * 128] = cb[
                4 * c + nl
            ]
    full = np.ascontiguousarray(full.transpose(1, 0, 2))  # [128, NCHUNK, 512]
    hi = full.astype(bf16)
    lo = (full - hi.astype(np.float32)).astype(bf16)
    return np.stack([hi, lo])


def kernel(x: np.ndarray, codebook: np.ndarray) -> np.ndarray:
    from concourse.bass_utils import run_bass_kernel_spmd

    x = np.ascontiguousarray(np.asarray(x, dtype=np.float32))
    cb = np.ascontiguousarray(np.asarray(codebook, dtype=np.float32))
    g = _gumbel()
    cbd = _blockdiag_cb(cb)

    nc = _build_bass()
    bpc = B // NCORES
    in_maps = []
    for i in range(NCORES):
        in_maps.append(
            {
                "x": x[i * bpc : (i + 1) * bpc].reshape(TOK, N * S),
                "g": g[i * bpc : (i + 1) * bpc].reshape(TOK, N * S),
                "cbd": cbd,
            }
        )
    res = run_bass_kernel_spmd(nc, in_maps, list(range(NCORES)))
    out = np.concatenate(
        [r["out"].reshape(bpc, T, N * E) for r in res.results], axis=0
    )
    return out


# revision 31
# speedup vs baseline: 1.0896x; 1.0896x over previous
"""Trainium2 Bass kernel for nn_CategoricalEncoder (vq_codebook).

Computes: logits = x.reshape(B,T,N,S); idx = argmax(logits + gumbel(key42));
out[b,t,n,:] = codebook[n, idx[b,t,n], :]  (the straight-through softmax terms
cancel numerically to ~1e-7, so the exact one-hot @ codebook matmul suffices).

The Gumbel noise is a fixed constant (key 42, fixed shape/dtype); it is
precomputed on the host with the same jax backend the reference uses and
streamed to the device, where argmax/one-hot/codebook-matmul run.

Sharding: data-parallel over batch B across the 8 NeuronCores; codebook
replicated.
"""

import numpy as np

B, T, N, S, E = 32, 256, 32, 32, 128
NCORES = 8
TOK = (B // NCORES) * T  # tokens per core (1024)
PTILE = 128
NTILES = TOK // PTILE  # 8
NCHUNK = 8  # (n,s) chunks of 128 per token-tile; each covers 4 n's

_cache: dict = {}


def _gumbel() -> np.ndarray:
    """Gumbel(0,1) noise bits exactly as jax.random.categorical(key(42), ...)
    draws them on this process's default jax backend."""
    if "g" not in _cache:
        import jax
        import jax.numpy as jnp

        g = jax.random.gumbel(jax.random.key(42), (B, T, N, S), jnp.float32)
        _cache["g"] = np.asarray(g).reshape(B, T, N * S)
    return _cache["g"]


def _build_bass():
    if "nc" in _cache:
        return _cache["nc"]
    from contextlib import ExitStack

    import concourse.bacc as bacc
    import concourse.bass as bass
    import concourse.tile as tile
    from concourse import mybir
    from concourse.masks import make_identity

    fp32 = mybir.dt.float32
    bf16 = mybir.dt.bfloat16
    nc = bacc.Bacc("TRN2", target_bir_lowering=False)
    x = nc.declare_dram_parameter("x", [TOK, N * S], fp32, isOutput=False)
    g = nc.declare_dram_parameter("g", [TOK, N * S], fp32, isOutput=False)
    cbd = nc.declare_dram_parameter("cbd", [2, 128, NCHUNK, 512], bf16, isOutput=False)
    out = nc.declare_dram_parameter("out", [TOK, N * E], fp32, isOutput=True)

    with ExitStack() as ctx:
        tc = ctx.enter_context(tile.TileContext(nc))
        singles = ctx.enter_context(tc.tile_pool(name="singles", bufs=1))
        ins = ctx.enter_context(tc.tile_pool(name="ins", bufs=3))
        work = ctx.enter_context(tc.tile_pool(name="work", bufs=3))
        oht_pool = ctx.enter_context(tc.tile_pool(name="oht", bufs=4))
        stage_pool = ctx.enter_context(tc.tile_pool(name="stage", bufs=2))
        pst = ctx.enter_context(tc.tile_pool(name="pst", bufs=2, space="PSUM"))
        pso = ctx.enter_context(tc.tile_pool(name="pso", bufs=3, space="PSUM"))

        identity = singles.tile([128, 128], fp32, tag="identity")
        make_identity(nc, identity)

        # Block-diagonal codebook: chunk c is [(4n x 32s)=128, (4n x 128e)=512]
        # so a single K=128 matmul contracts 4 consecutive n's at once.
        # bf16 hi+lo split: out = onehot @ hi + onehot @ lo (error ~2^-18).
        # Loaded via gpsimd (SWDGE) so the SP HWDGE ring starts with x/g.
        bdcb = []
        cbd_insts = []
        for h in range(2):
            t_ = singles.tile([128, NCHUNK, 512], bf16, tag=f"bdcb{h}")
            cbd_insts.append(nc.gpsimd.dma_start(out=t_, in_=cbd[h]).ins)
            bdcb.append(t_)

        # Chain input DMAs in consumption order so tile0's data lands first
        # at full HBM bandwidth instead of all transfers time-sharing it.
        in_chain = []

        for it in range(NTILES):
            rows = slice(it * PTILE, (it + 1) * PTILE)
            x_t = ins.tile([PTILE, N * S], fp32, tag="x")
            g_t = ins.tile([PTILE, N * S], fp32, tag="g")
            in_chain.append(nc.sync.dma_start(out=x_t, in_=x[rows]).ins)
            in_chain.append(nc.sync.dma_start(out=g_t, in_=g[rows]).ins)
            if it == 0:
                # codebook transfers slot in right after tile0's inputs
                in_chain.extend(cbd_insts)

            v = work.tile([PTILE, N * S], fp32, tag="v")
            nc.vector.tensor_tensor(out=v, in0=x_t, in1=g_t, op=mybir.AluOpType.add)
            v3 = v.rearrange("p (n s) -> p n s", s=S)

            m = work.tile([PTILE, N], fp32, tag="m")
            nc.vector.tensor_reduce(
                out=m, in_=v3, axis=mybir.AxisListType.X, op=mybir.AluOpType.max
            )

            onehot = work.tile([PTILE, N * S], fp32, tag="onehot")
            m_b = m.unsqueeze(2).broadcast_to([PTILE, N, S])
            nc.vector.tensor_tensor(
                out=onehot.rearrange("p (n s) -> p n s", s=S),
                in0=v3,
                in1=m_b,
                op=mybir.AluOpType.is_ge,
            )

            stage = stage_pool.tile([PTILE, N * E], fp32, tag="stage")
            # 4 transposes share one PSUM bank -> single ACT copy (fp32->bf16)
            ohTs = []
            for q in range(2):
                pst_t = pst.tile([128, 512], fp32, tag="pst")
                for c4 in range(4):
                    c = 4 * q + c4
                    nc.tensor.transpose(
                        pst_t[:, c4 * 128 : (c4 + 1) * 128],
                        onehot[:, c * 128 : (c + 1) * 128],
                        identity,
                    )
                ohT = oht_pool.tile([128, 512], bf16, tag="ohT")
                nc.scalar.copy(out=ohT, in_=pst_t)
                ohTs.append(ohT)
            for p2 in range(4):  # pairs of chunks -> one 2-bank psum tile
                pso_t = pso.tile([128, 1024], fp32, tag="pso")
                for ci in range(2):
                    c = 2 * p2 + ci
                    ohT_c = ohTs[c // 4][:, (c % 4) * 128 : (c % 4 + 1) * 128]
                    ps = pso_t[:, ci * 512 : (ci + 1) * 512]
                    nc.tensor.matmul(ps, ohT_c, bdcb[0][:, c, :], start=True, stop=False)
                    nc.tensor.matmul(ps, ohT_c, bdcb[1][:, c, :], start=False, stop=True)
                # Split PSUM->SBUF evacuation between DVE and ACT
                eng_dve = ((it * 4 + p2) * 5) % 16 < 5  # ~5/16 of copies on DVE
                dst = stage[:, p2 * 1024 : (p2 + 1) * 1024]
                if eng_dve:
                    nc.vector.tensor_copy(out=dst, in_=pso_t)
                else:
                    nc.scalar.copy(out=dst, in_=pso_t)
                if p2 == 1:
                    nc.scalar.dma_start(
                        out=out[rows, :2048], in_=stage[:, :2048]
                    )
            # Output store on the ACT HWDGE ring so it doesn't FIFO-serialize
            # with the input loads on the SP ring.
            nc.scalar.dma_start(out=out[rows, 2048:], in_=stage[:, 2048:])

        # Stride-3 chains: ~3 input transfers in flight — early tiles get
        # most of the HBM bandwidth, completion latency stays hidden.
        for j, inst_ in enumerate(in_chain):
            tc.chain_iter_dep(f"in_dma_chain{j % 3}", inst_)

    nc.finalize()
    _cache["nc"] = nc
    return nc


def _blockdiag_cb(cb: np.ndarray) -> np.ndarray:
    """[N,S,E] -> [2, 128, NCHUNK, 512] bf16 block-diagonal hi/lo tiles."""
    import ml_dtypes

    bf16 = ml_dtypes.bfloat16
    full = np.zeros((NCHUNK, 128, 512), dtype=np.float32)
    for c in range(NCHUNK):
        for nl in range(4):
            full[c, nl * 32 : (nl + 1) * 32, nl * 128 : (nl + 1) * 128] = cb[
                4 * c + nl
            ]
    full = np.ascontiguousarray(full.transpose(1, 0, 2))  # [128, NCHUNK, 512]
    hi = full.astype(bf16)
    lo = (full - hi.astype(np.float32)).astype(bf16)
    return np.stack([hi, lo])


def kernel(x: np.ndarray, codebook: np.ndarray) -> np.ndarray:
    from concourse.bass_utils import run_bass_kernel_spmd

    x = np.ascontiguousarray(np.asarray(x, dtype=np.float32))
    cb = np.ascontiguousarray(np.asarray(codebook, dtype=np.float32))
    g = _gumbel()
    cbd = _blockdiag_cb(cb)

    nc = _build_bass()
    bpc = B // NCORES
    in_maps = []
    for i in range(NCORES):
        in_maps.append(
            {
                "x": x[i * bpc : (i + 1) * bpc].reshape(TOK, N * S),
                "g": g[i * bpc : (i + 1) * bpc].reshape(TOK, N * S),
                "cbd": cbd,
            }
        )
    res = run_bass_kernel_spmd(nc, in_maps, list(range(NCORES)))
    out = np.concatenate(
        [r["out"].reshape(bpc, T, N * E) for r in res.results], axis=0
    )
    return out


# revision 35
# speedup vs baseline: 1.2191x; 1.1189x over previous
"""Trainium2 Bass kernel for nn_CategoricalEncoder (vq_codebook).

Computes: logits = x.reshape(B,T,N,S); idx = argmax(logits + gumbel(key42));
out[b,t,n,:] = codebook[n, idx[b,t,n], :]  (the straight-through softmax terms
cancel numerically to ~1e-7, so the exact one-hot @ codebook matmul suffices).

The Gumbel noise is a fixed constant (key 42, fixed shape/dtype); it is
precomputed on the host with the same jax backend the reference uses and
streamed to the device, where argmax/one-hot/codebook-matmul run.

Sharding: data-parallel over batch B across the 8 NeuronCores; codebook
replicated.
"""

import numpy as np

B, T, N, S, E = 32, 256, 32, 32, 128
NCORES = 8
TOK = (B // NCORES) * T  # tokens per core (1024)
PTILE = 128
NTILES = TOK // PTILE  # 8
NCHUNK = 8  # (n,s) chunks of 128 per token-tile; each covers 4 n's

_cache: dict = {}


def _gumbel() -> np.ndarray:
    """Gumbel(0,1) noise bits exactly as jax.random.categorical(key(42), ...)
    draws them on this process's default jax backend."""
    if "g" not in _cache:
        import jax
        import jax.numpy as jnp

        g = jax.random.gumbel(jax.random.key(42), (B, T, N, S), jnp.float32)
        _cache["g"] = np.asarray(g).reshape(B, T, N * S)
    return _cache["g"]


def _build_bass():
    if "nc" in _cache:
        return _cache["nc"]
    from contextlib import ExitStack

    import concourse.bacc as bacc
    import concourse.bass as bass
    import concourse.tile as tile
    from concourse import mybir
    from concourse.masks import make_identity

    fp32 = mybir.dt.float32
    bf16 = mybir.dt.bfloat16
    nc = bacc.Bacc("TRN2", target_bir_lowering=False)
    xg = nc.declare_dram_parameter("xg", [TOK, 2 * N * S], fp32, isOutput=False)
    cbd = nc.declare_dram_parameter("cbd", [2, 128, NCHUNK, 512], bf16, isOutput=False)
    out = nc.declare_dram_parameter("out", [TOK, N * E], fp32, isOutput=True)

    with ExitStack() as ctx:
        tc = ctx.enter_context(tile.TileContext(nc))
        singles = ctx.enter_context(tc.tile_pool(name="singles", bufs=1))
        ins = ctx.enter_context(tc.tile_pool(name="ins", bufs=3))
        work = ctx.enter_context(tc.tile_pool(name="work", bufs=3))
        oht_pool = ctx.enter_context(tc.tile_pool(name="oht", bufs=4))
        stage_pool = ctx.enter_context(tc.tile_pool(name="stage", bufs=2))
        pst = ctx.enter_context(tc.tile_pool(name="pst", bufs=2, space="PSUM"))
        pso = ctx.enter_context(tc.tile_pool(name="pso", bufs=3, space="PSUM"))

        identity = singles.tile([128, 128], fp32, tag="identity")
        make_identity(nc, identity)

        # Block-diagonal codebook: chunk c is [(4n x 32s)=128, (4n x 128e)=512]
        # so a single K=128 matmul contracts 4 consecutive n's at once.
        # bf16 hi+lo split: out = onehot @ hi + onehot @ lo (error ~2^-18).
        # Loaded via gpsimd (SWDGE) so the SP HWDGE ring starts with x/g.
        bdcb = []
        cbd_insts = []
        for h in range(2):
            t_ = singles.tile([128, NCHUNK, 512], bf16, tag=f"bdcb{h}")
            cbd_insts.append(nc.gpsimd.dma_start(out=t_, in_=cbd[h]).ins)
            bdcb.append(t_)

        # Chain input DMAs in consumption order so tile0's data lands first
        # at full HBM bandwidth instead of all transfers time-sharing it.
        in_chain = []

        for it in range(NTILES):
            rows = slice(it * PTILE, (it + 1) * PTILE)
            xg_t = ins.tile([PTILE, 2 * N * S], fp32, tag="xg")
            in_chain.append(nc.sync.dma_start(out=xg_t, in_=xg[rows]).ins)
            if it == 0:
                # codebook transfers slot in right after tile0's inputs
                in_chain.extend(cbd_insts)
            x_t = xg_t[:, : N * S]
            g_t = xg_t[:, N * S :]

            v = work.tile([PTILE, N * S], fp32, tag="v")
            nc.vector.tensor_tensor(out=v, in0=x_t, in1=g_t, op=mybir.AluOpType.add)
            v3 = v.rearrange("p (n s) -> p n s", s=S)

            m = work.tile([PTILE, N], fp32, tag="m")
            nc.vector.tensor_reduce(
                out=m, in_=v3, axis=mybir.AxisListType.X, op=mybir.AluOpType.max
            )

            onehot = work.tile([PTILE, N * S], fp32, tag="onehot")
            m_b = m.unsqueeze(2).broadcast_to([PTILE, N, S])
            nc.vector.tensor_tensor(
                out=onehot.rearrange("p (n s) -> p n s", s=S),
                in0=v3,
                in1=m_b,
                op=mybir.AluOpType.is_ge,
            )

            stage = stage_pool.tile([PTILE, N * E], fp32, tag="stage")
            # 4 transposes share one PSUM bank -> single ACT copy (fp32->bf16)
            ohTs = []
            for q in range(2):
                pst_t = pst.tile([128, 512], fp32, tag="pst")
                for c4 in range(4):
                    c = 4 * q + c4
                    nc.tensor.transpose(
                        pst_t[:, c4 * 128 : (c4 + 1) * 128],
                        onehot[:, c * 128 : (c + 1) * 128],
                        identity,
                    )
                ohT = oht_pool.tile([128, 512], bf16, tag="ohT")
                nc.scalar.copy(out=ohT, in_=pst_t)
                ohTs.append(ohT)
            for p2 in range(4):  # pairs of chunks -> one 2-bank psum tile
                pso_t = pso.tile([128, 1024], fp32, tag="pso")
                for ci in range(2):
                    c = 2 * p2 + ci
                    ohT_c = ohTs[c // 4][:, (c % 4) * 128 : (c % 4 + 1) * 128]
                    ps = pso_t[:, ci * 512 : (ci + 1) * 512]
                    nc.tensor.matmul(ps, ohT_c, bdcb[0][:, c, :], start=True, stop=False)
                    nc.tensor.matmul(ps, ohT_c, bdcb[1][:, c, :], start=False, stop=True)
                # Split PSUM->SBUF evacuation between DVE and ACT
                eng_dve = ((it * 4 + p2) * 5) % 16 < 5  # ~5/16 of copies on DVE
                dst = stage[:, p2 * 1024 : (p2 + 1) * 1024]
                if eng_dve:
                    nc.vector.tensor_copy(out=dst, in_=pso_t)
                else:
                    nc.scalar.copy(out=dst, in_=pso_t)
                if p2 == 1:
                    nc.scalar.dma_start(
                        out=out[rows, :2048], in_=stage[:, :2048]
                    )
            # Output store on the ACT HWDGE ring so it doesn't FIFO-serialize
            # with the input loads on the SP ring.
            nc.scalar.dma_start(out=out[rows, 2048:], in_=stage[:, 2048:])

        # Stride-3 chains: ~3 input transfers in flight — early tiles get
        # most of the HBM bandwidth, completion latency stays hidden.
        for j, inst_ in enumerate(in_chain):
            tc.chain_iter_dep(f"in_dma_chain{j % 3}", inst_)

    nc.finalize()
    _cache["nc"] = nc
    return nc


def _blockdiag_cb(cb: np.ndarray) -> np.ndarray:
    """[N,S,E] -> [2, 128, NCHUNK, 512] bf16 block-diagonal hi/lo tiles."""
    import ml_dtypes

    bf16 = ml_dtypes.bfloat16
    full = np.zeros((NCHUNK, 128, 512), dtype=np.float32)
    for c in range(NCHUNK):
        for nl in range(4):
            full[c, nl * 32 : (nl + 1) * 32, nl * 128 : (nl + 1) * 128] = cb[
                4 * c + nl
            ]
    full = np.ascontiguousarray(full.transpose(1, 0, 2))  # [128, NCHUNK, 512]
    hi = full.astype(bf16)
    lo = (full - hi.astype(np.float32)).astype(bf16)
    return np.stack([hi, lo])


def _in_maps(x: np.ndarray, codebook: np.ndarray) -> list:
    x = np.ascontiguousarray(np.asarray(x, dtype=np.float32))
    cb = np.ascontiguousarray(np.asarray(codebook, dtype=np.float32))
    g = _gumbel()
    cbd = _blockdiag_cb(cb)
    bpc = B // NCORES
    in_maps = []
    for i in range(NCORES):
        xs = x[i * bpc : (i + 1) * bpc].reshape(TOK, N * S)
        gs = g[i * bpc : (i + 1) * bpc].reshape(TOK, N * S)
        in_maps.append(
            {
                "xg": np.concatenate([xs, gs], axis=1),
                "cbd": cbd,
            }
        )
    return in_maps


def kernel(x: np.ndarray, codebook: np.ndarray) -> np.ndarray:
    from concourse.bass_utils import run_bass_kernel_spmd

    nc = _build_bass()
    in_maps = _in_maps(x, codebook)
    bpc = B // NCORES
    res = run_bass_kernel_spmd(nc, in_maps, list(range(NCORES)))
    out = np.concatenate(
        [r["out"].reshape(bpc, T, N * E) for r in res.results], axis=0
    )
    return out
